# revision 9
# baseline (speedup 1.0000x reference)
"""Trainium2 Bass kernel for nn_ChatDecoder: greedy LSTM decoder, 32 steps.

Strategy (8 NeuronCores, SPMD):
  - Vocab-sharded dense projection: each core holds W_dense[:, c*4000:(c+1)*4000]
    resident in SBUF and computes logits [64, 4000] for its shard each step.
  - Unit-sharded LSTM: each core computes 128 of the 1024 hidden units
    (all 4 gates), then an AllGather assembles the full h state on every core.
  - Greedy argmax: per-core top-1 via DVE max/max_index, then a tiny AllGather
    of (value, global index) pairs + local combine -> global argmax on every
    core (first-occurrence tie-break, matching jnp.argmax).
  - Next-token embedding: indirect DMA gather emb[idx] -> PE transpose to
    [E-on-partitions, batch] layout for the next step's matmuls.
  - All matmuls fp32 (PE hi/lo path); gates via tanh only:
    sigmoid(x) = 0.5 + 0.5*tanh(0.5 x) (tanh has a much tighter ULP budget
    than the sigmoid table, and greedy decode is numerically fragile here).

Layouts (host-prepared, per core c):
  h.T blocks   : [128, 8*64]  block u at [:, 64u:64u+64] = h.T[128u:128(u+1), :]
  x.T blocks   : [128, 4*64]  same scheme over E=512
  Wz (z.T stationary tiles): [128, 12*512], K-chunk k at [:, 512k:512k+512],
                 gate-block m (i,f,g,o of this core's units) at +128m
  Wd (dense moving tiles)  : [128, 8*4000], K-chunk k at [:, 4000k:4000k+4000]
Output per core: [32, 64, 4000] (step, batch, vocab-shard); host reassembles.
"""

import sys
from contextlib import ExitStack

import numpy as np

for _p in ("/opt/trn_rl_repo",):
    if _p not in sys.path:
        sys.path.insert(0, _p)

import concourse.bass as bass
import concourse.tile as tile
from concourse import bacc, mybir
from concourse.bass_utils import run_bass_kernel_spmd

F32 = mybir.dt.float32
I32 = mybir.dt.int32
U32 = mybir.dt.uint32
TANH = mybir.ActivationFunctionType.Tanh
OP = mybir.AluOpType

V, E, U, B, T_FULL = 32000, 512, 1024, 64, 32
NC = 8
VS = V // NC          # 4000 vocab shard
NT = 500              # dense moving tile (<=512 fp32)
NTILES = VS // NT     # 8
KD = U // 128         # 8 dense K-chunks
KZ = (E + U) // 128   # 12 z K-chunks
GO = 1
RG = [list(range(NC))]
BIG = 1.0e9


def build_program(T: int = T_FULL, has_bd: bool = False, has_bg: bool = False):
    nc = bacc.Bacc(
        "TRN2", target_bir_lowering=False, debug=False, num_devices=NC
    )

    def inp(name, shape, dtype=F32):
        return nc.dram_tensor(name, list(shape), dtype, kind="ExternalInput")

    h0t = inp("h0t", (128, 8 * 64))
    c0t = inp("c0t", (128, 64))
    emb_d = inp("emb", (V, E))
    x0t = inp("x0t", (128, 4 * 64))
    wz_d = inp("wz", (128, KZ * 512))
    wd_d = inp("wd", (128, KD * VS))
    bg_d = inp("bgate", (128, 4))
    vo_d = inp("vocoff", (64, 1))
    id_d = inp("ident", (64, 64))
    if has_bd:
        bd_d = inp("bd", (64, VS))
    out_d = nc.dram_tensor("out", [T, B, VS], F32, kind="ExternalOutput")

    with tile.TileContext(nc) as tc, ExitStack() as ctx:
        const = ctx.enter_context(tc.tile_pool(name="const", bufs=1))
        hpool = ctx.enter_context(tc.tile_pool(name="hpool", bufs=2))
        cpool = ctx.enter_context(tc.tile_pool(name="cpool", bufs=2))
        xtpool = ctx.enter_context(tc.tile_pool(name="xtpool", bufs=2))
        xpool = ctx.enter_context(tc.tile_pool(name="xpool", bufs=2))
        gates = ctx.enter_context(tc.tile_pool(name="gates", bufs=2))
        lpool = ctx.enter_context(tc.tile_pool(name="lpool", bufs=1))
        ampool = ctx.enter_context(tc.tile_pool(name="ampool", bufs=2))
        dram = ctx.enter_context(tc.tile_pool(name="dram", bufs=2, space="DRAM"))
        zpsum = ctx.enter_context(tc.tile_pool(name="zpsum", bufs=2, space="PSUM"))
        dpsum = ctx.enter_context(tc.tile_pool(name="dpsum", bufs=3, space="PSUM"))
        tpsum = ctx.enter_context(tc.tile_pool(name="tpsum", bufs=2, space="PSUM"))

        wz = const.tile([128, KZ * 512], F32)
        nc.sync.dma_start(wz[:], wz_d[:])
        wd = const.tile([128, KD * VS], F32)
        nc.sync.dma_start(wd[:], wd_d[:])
        bg = const.tile([128, 4], F32)
        nc.sync.dma_start(bg[:], bg_d[:])
        vo = const.tile([64, 1], F32)
        nc.sync.dma_start(vo[:], vo_d[:])
        idn = const.tile([64, 64], F32)
        nc.sync.dma_start(idn[:], id_d[:])
        if has_bd:
            bd = const.tile([64, VS], F32)
            nc.sync.dma_start(bd[:], bd_d[:])

        h_cur = hpool.tile([128, 8 * 64], F32, name="h_sb")
        nc.sync.dma_start(h_cur[:], h0t[:])
        c_cur = cpool.tile([128, 64], F32, name="c_sb")
        nc.sync.dma_start(c_cur[:], c0t[:])
        xt_cur = xtpool.tile([128, 4 * 64], F32, name="xt_sb")
        nc.sync.dma_start(xt_cur[:], x0t[:])

        # fp32 self-loading matmuls tolerate only one sync wait. Make the PE
        # observe each DMA-loaded tensor it will read via a tiny dummy matmul
        # (one wait each); subsequent matmuls then inherit those clocks.
        warm = ctx.enter_context(tc.tile_pool(name="warm", bufs=1, space="PSUM"))
        wps = warm.tile([1, 1], F32, name="wps")
        for src in (wz, wd, idn, h_cur, xt_cur):
            nc.tensor.matmul(
                wps[:], lhsT=src[0:32, 0:1], rhs=src[0:32, 0:1],
                start=True, stop=True, skip_group_check=True,
            )

        # z-chunk order: h chunks first (ready early), x chunks last
        korder = list(range(4, 12)) + list(range(4))

        for t in range(T):
            # ---- z.T = Wz.T @ [x; h] for this core's 128 units x 4 gates ----
            zps = zpsum.tile([128, 4 * 64], F32, name="zps")
            for m in range(4):
                for ki, k in enumerate(korder):
                    if k >= 4:
                        rhs = h_cur[:, 64 * (k - 4) : 64 * (k - 3)]
                    else:
                        rhs = xt_cur[:, 64 * k : 64 * (k + 1)]
                    nc.tensor.matmul(
                        zps[:, 64 * m : 64 * (m + 1)],
                        lhsT=wz[:, 512 * k + 128 * m : 512 * k + 128 * (m + 1)],
                        rhs=rhs,
                        start=(ki == 0),
                        stop=(ki == KZ - 1),
                    )

            # ---- LSTM cell elementwise (gate order i,f,g,o at m=0..3) ----
            # sigmoid(x) = 0.5 + 0.5 tanh(x/2); biases folded into ACT bias.
            def act_gate(name, m, scale):
                tl = gates.tile([128, 64], F32, name=name)
                kw = {}
                if has_bg:
                    kw["bias"] = bg[:, m : m + 1]
                nc.scalar.activation(
                    tl[:], zps[:, 64 * m : 64 * (m + 1)], TANH, scale=scale, **kw
                )
                return tl

            ti = act_gate("ti", 0, 0.5)
            tf = act_gate("tf", 1, 0.5)
            tg = act_gate("tg", 2, 1.0)
            to = act_gate("to", 3, 0.5)

            sf = gates.tile([128, 64], F32, name="sf")  # sigmoid(f) = 0.5+0.5*tf
            nc.vector.tensor_scalar(sf[:], tf[:], 0.5, 0.5, OP.mult, OP.add)
            si = gates.tile([128, 64], F32, name="si")
            nc.vector.tensor_scalar(si[:], ti[:], 0.5, 0.5, OP.mult, OP.add)
            so = gates.tile([128, 64], F32, name="so")
            nc.vector.tensor_scalar(so[:], to[:], 0.5, 0.5, OP.mult, OP.add)
            q1 = gates.tile([128, 64], F32, name="q1")  # sig(f)*c
            nc.vector.tensor_mul(q1[:], sf[:], c_cur[:])
            q2 = gates.tile([128, 64], F32, name="q2")  # sig(i)*tanh(g)
            nc.vector.tensor_mul(q2[:], si[:], tg[:])
            c_new = cpool.tile([128, 64], F32, name="c_sb")
            nc.vector.tensor_add(c_new[:], q1[:], q2[:])
            c_cur = c_new
            tcn = gates.tile([128, 64], F32, name="tcn")  # tanh(c_new)
            nc.scalar.activation(tcn[:], c_new[:], TANH)
            hnew = gates.tile([128, 64], F32, name="hnew")
            nc.vector.tensor_mul(hnew[:], so[:], tcn[:])

            # ---- AllGather h slices -> full h.T on every core ----
            hsl = dram.tile([128, 64], F32, name="hsl")
            nc.sync.dma_start(hsl[:], hnew[:])
            hall = dram.tile([NC * 128, 64], F32, name="hall", addr_space="Shared")
            nc.gpsimd.collective_compute(
                "AllGather",
                OP.bypass,
                replica_groups=RG,
                ins=[hsl[:].opt()],
                outs=[hall[:].opt()],
            )
            h_new_sb = hpool.tile([128, 8 * 64], F32, name="h_sb")
            nc.sync.dma_start(
                h_new_sb[:].rearrange("p (u b) -> p u b", b=64),
                hall[:].rearrange("(u p) b -> p u b", p=128),
            )
            h_cur = h_new_sb

            # ---- dense: logits[64, 4000] = h.T^T @ Wd (+ b_dense) ----
            logits = lpool.tile([64, VS], F32, name="logits")
            for n in range(NTILES):
                dps = dpsum.tile([64, NT], F32, name="dps")
                for k in range(KD):
                    nc.tensor.matmul(
                        dps[:],
                        lhsT=h_cur[:, 64 * k : 64 * (k + 1)],
                        rhs=wd[:, VS * k + NT * n : VS * k + NT * (n + 1)],
                        start=(k == 0),
                        stop=(k == KD - 1),
                    )
                if has_bd:
                    nc.vector.tensor_add(
                        logits[:, NT * n : NT * (n + 1)], dps[:], bd[:, NT * n : NT * (n + 1)]
                    )
                else:
                    nc.vector.tensor_copy(logits[:, NT * n : NT * (n + 1)], dps[:])

            nc.sync.dma_start(out_d[t], logits[:])

            if t == T - 1:
                break

            # ---- local top-1 + global argmax combine ----
            lmax = ampool.tile([64, 8], F32, name="lmax")
            nc.vector.max(out=lmax[:], in_=logits[:])
            lidx = ampool.tile([64, 8], U32, name="lidx")
            nc.vector.max_index(lidx[:], lmax[:], logits[:])
            idxf = ampool.tile([64, 1], F32, name="idxf")
            nc.vector.tensor_copy(idxf[:], lidx[:, 0:1])
            pk = ampool.tile([64, 2], F32, name="pk")
            nc.vector.tensor_copy(pk[:, 0:1], lmax[:, 0:1])
            nc.vector.tensor_add(pk[:, 1:2], idxf[:], vo[:])

            amin = dram.tile([64, 2], F32, name="amin")
            nc.sync.dma_start(amin[:], pk[:])
            amout = dram.tile([NC * 64, 2], F32, name="amout", addr_space="Shared")
            nc.gpsimd.collective_compute(
                "AllGather",
                OP.bypass,
                replica_groups=RG,
                ins=[amin[:].opt()],
                outs=[amout[:].opt()],
            )
            cand = ampool.tile([64, 16], F32, name="cand")
            nc.sync.dma_start(
                cand[:].rearrange("b (c j) -> b c j", j=2),
                amout[:].rearrange("(c b) j -> b c j", c=NC),
            )
            c3 = cand[:].rearrange("b (c j) -> b c j", j=2)
            vals = c3[:, :, 0]
            idxs = c3[:, :, 1]
            gmx = ampool.tile([64, 1], F32, name="gmx")
            nc.vector.tensor_reduce(gmx[:], vals, axis=mybir.AxisListType.X, op=OP.max)
            eq = ampool.tile([64, 8], U32, name="eq")
            nc.vector.tensor_tensor(
                out=eq[:], in0=vals, in1=gmx[:].to_broadcast([64, 8]), op=OP.is_equal
            )
            pick = ampool.tile([64, 8], F32, name="pick")
            nc.vector.memset(pick[:], BIG)
            nc.vector.copy_predicated(pick[:], eq[:], idxs)
            gixf = ampool.tile([64, 1], F32, name="gixf")
            nc.vector.tensor_reduce(gixf[:], pick[:], axis=mybir.AxisListType.X, op=OP.min)
            gi32 = ampool.tile([64, 1], I32, name="gi32")
            nc.vector.tensor_copy(gi32[:], gixf[:])

            # ---- gather emb[idx] and transpose to [E-part, batch] blocks ----
            xr = xpool.tile([64, E], F32, name="xr")
            nc.gpsimd.indirect_dma_start(
                out=xr[:],
                out_offset=None,
                in_=emb_d[:],
                in_offset=bass.IndirectOffsetOnAxis(ap=gi32[:, :1], axis=0),
            )
            xt_next = xtpool.tile([128, 4 * 64], F32, name="xt_sb")
            for j in range(4):
                tps = tpsum.tile([128, 64], F32, name="tps")
                nc.tensor.transpose(
                    tps[:], xr[:, 128 * j : 128 * (j + 1)], idn[:]
                )
                nc.vector.tensor_copy(xt_next[:, 64 * j : 64 * (j + 1)], tps[:])
            xt_cur = xt_next

    nc.compile()
    return nc


def make_in_maps(inputs: dict, T: int = T_FULL):
    h0 = np.ascontiguousarray(np.asarray(inputs["h0"], np.float32))
    c0 = np.ascontiguousarray(np.asarray(inputs["c0"], np.float32))
    emb = np.ascontiguousarray(np.asarray(inputs["emb"], np.float32))
    W_ih = np.asarray(inputs["W_ih"], np.float32)
    W_hh = np.asarray(inputs["W_hh"], np.float32)
    b = np.asarray(inputs["b"], np.float32)
    W_d = np.asarray(inputs["W_dense"], np.float32)
    b_d = np.asarray(inputs["b_dense"], np.float32)

    has_bd = bool(np.any(b_d != 0))
    has_bg = bool(np.any(b != 0))

    Wz_full = np.concatenate([W_ih, W_hh], axis=0)  # [1536, 4096]
    h0t = np.ascontiguousarray(
        h0.T.reshape(8, 128, 64).transpose(1, 0, 2).reshape(128, 512)
    )
    x0 = emb[GO]  # [512]
    x0t = np.ascontiguousarray(
        np.repeat(x0[:, None], B, axis=1).reshape(4, 128, 64).transpose(1, 0, 2).reshape(128, 256)
    )
    ident = np.eye(64, dtype=np.float32)

    in_maps = []
    for c in range(NC):
        ucols = np.concatenate(
            [np.arange(g * U + 128 * c, g * U + 128 * (c + 1)) for g in range(4)]
        )
        Wz_c = Wz_full[:, ucols]  # [1536, 512] cols: 128 i, 128 f, 128 g, 128 o
        # reorder cols so gate-block m at [:,128m:128(m+1)] -> already true
        wz_l = np.ascontiguousarray(
            Wz_c.reshape(KZ, 128, 512).transpose(1, 0, 2).reshape(128, KZ * 512)
        )
        Wd_c = W_d[:, VS * c : VS * (c + 1)]  # [1024, 4000]
        wd_l = np.ascontiguousarray(
            Wd_c.reshape(KD, 128, VS).transpose(1, 0, 2).reshape(128, KD * VS)
        )
        bgate = np.zeros((128, 4), np.float32)
        for g, scale in zip(range(4), (0.5, 0.5, 1.0, 0.5)):
            bgate[:, g] = b[g * U + 128 * c : g * U + 128 * (c + 1)] * scale
        c0t = np.ascontiguousarray(c0[:, 128 * c : 128 * (c + 1)].T)
        vocoff = np.full((64, 1), VS * c, np.float32)
        m = {
            "h0t": h0t,
            "c0t": c0t,
            "emb": emb,
            "x0t": x0t,
            "wz": wz_l,
            "wd": wd_l,
            "bgate": bgate,
            "vocoff": vocoff,
            "ident": ident,
        }
        if has_bd:
            m["bd"] = np.ascontiguousarray(
                np.repeat(b_d[VS * c : VS * (c + 1)][None, :], B, axis=0)
            )
        in_maps.append(m)
    return in_maps, has_bd, has_bg


def assemble_output(results, T: int = T_FULL):
    parts = [np.asarray(r["out"]).reshape(T, B, VS) for r in results]
    full = np.concatenate(parts, axis=2)  # [T, 64, 32000]
    return np.ascontiguousarray(full.transpose(1, 0, 2))  # [64, T, 32000]


def kernel(**inputs) -> np.ndarray:
    in_maps, has_bd, has_bg = make_in_maps(inputs)
    nc = build_program(T_FULL, has_bd=has_bd, has_bg=has_bg)
    res = run_bass_kernel_spmd(nc, in_maps, core_ids=list(range(NC)))
    return assemble_output(res.results)


if __name__ == "__main__":
    # smoke: random small check against numpy oracle is in test.py
    print("kernel module OK")


# revision 14
# speedup vs baseline: 1.1759x; 1.1759x over previous
"""Trainium2 Bass kernel for nn_ChatDecoder: greedy LSTM decoder, 32 steps.

Strategy (8 NeuronCores, SPMD):
  - Vocab-sharded dense projection: each core holds W_dense[:, c*4000:(c+1)*4000]
    resident in SBUF and computes logits [64, 4000] for its shard each step.
  - Unit-sharded LSTM: each core computes 128 of the 1024 hidden units
    (all 4 gates), then an AllGather assembles the full h state on every core.
  - Greedy argmax: per-core top-1 via DVE max/max_index, then a tiny AllGather
    of (value, global index) pairs + local combine -> global argmax on every
    core (first-occurrence tie-break, matching jnp.argmax).
  - Next-token embedding: indirect DMA gather emb[idx] -> PE transpose to
    [E-on-partitions, batch] layout for the next step's matmuls.
  - All matmuls fp32 (PE hi/lo path); gates via tanh only:
    sigmoid(x) = 0.5 + 0.5*tanh(0.5 x) (tanh has a much tighter ULP budget
    than the sigmoid table, and greedy decode is numerically fragile here).

Layouts (host-prepared, per core c):
  h.T blocks   : [128, 8*64]  block u at [:, 64u:64u+64] = h.T[128u:128(u+1), :]
  x.T blocks   : [128, 4*64]  same scheme over E=512
  Wz (z.T stationary tiles): [128, 12*512], K-chunk k at [:, 512k:512k+512],
                 gate-block m (i,f,g,o of this core's units) at +128m
  Wd (dense moving tiles)  : [128, 8*4000], K-chunk k at [:, 4000k:4000k+4000]
Output per core: [32, 64, 4000] (step, batch, vocab-shard); host reassembles.
"""

import sys
from contextlib import ExitStack

import numpy as np

for _p in ("/opt/trn_rl_repo",):
    if _p not in sys.path:
        sys.path.insert(0, _p)

import concourse.bass as bass
import concourse.tile as tile
from concourse import bacc, mybir
from concourse.bass_utils import run_bass_kernel_spmd

F32 = mybir.dt.float32
I32 = mybir.dt.int32
U32 = mybir.dt.uint32
TANH = mybir.ActivationFunctionType.Tanh
OP = mybir.AluOpType

V, E, U, B, T_FULL = 32000, 512, 1024, 64, 32
NC = 8
VS = V // NC          # 4000 vocab shard
NT = 500              # dense moving tile (<=512 fp32)
NTILES = VS // NT     # 8
KD = U // 128         # 8 dense K-chunks
KZ = (E + U) // 128   # 12 z K-chunks
GO = 1
RG = [list(range(NC))]
BIG = 1.0e9


def build_program(T: int = T_FULL, has_bd: bool = False, has_bg: bool = False):
    nc = bacc.Bacc(
        "TRN2", target_bir_lowering=False, debug=False, num_devices=NC
    )

    def inp(name, shape, dtype=F32):
        return nc.dram_tensor(name, list(shape), dtype, kind="ExternalInput")

    h0t = inp("h0t", (128, 8 * 64))
    c0t = inp("c0t", (128, 64))
    emb_d = inp("emb", (V, E))
    x0t = inp("x0t", (128, 4 * 64))
    wz_d = inp("wz", (128, KZ * 512))
    wd_d = inp("wd", (128, KD * VS))
    bg_d = inp("bgate", (128, 4))
    vo_d = inp("vocoff", (64, 1))
    id_d = inp("ident", (64, 64))
    if has_bd:
        bd_d = inp("bd", (64, VS))
    out_d = nc.dram_tensor("out", [T, B, VS], F32, kind="ExternalOutput")

    with tile.TileContext(nc) as tc, ExitStack() as ctx:
        const = ctx.enter_context(tc.tile_pool(name="const", bufs=1))
        hpool = ctx.enter_context(tc.tile_pool(name="hpool", bufs=2))
        cpool = ctx.enter_context(tc.tile_pool(name="cpool", bufs=2))
        xtpool = ctx.enter_context(tc.tile_pool(name="xtpool", bufs=2))
        xpool = ctx.enter_context(tc.tile_pool(name="xpool", bufs=2))
        gates = ctx.enter_context(tc.tile_pool(name="gates", bufs=2))
        lpool = ctx.enter_context(tc.tile_pool(name="lpool", bufs=1))
        ampool = ctx.enter_context(tc.tile_pool(name="ampool", bufs=2))
        dram = ctx.enter_context(tc.tile_pool(name="dram", bufs=2, space="DRAM"))
        zpsum = ctx.enter_context(tc.tile_pool(name="zpsum", bufs=1, space="PSUM"))
        dpsum = ctx.enter_context(tc.tile_pool(name="dpsum", bufs=2, space="PSUM"))
        tpsum = ctx.enter_context(tc.tile_pool(name="tpsum", bufs=2, space="PSUM"))

        wz = const.tile([128, KZ * 512], F32)
        nc.sync.dma_start(wz[:], wz_d[:])
        wd = const.tile([128, KD * VS], F32)
        nc.sync.dma_start(wd[:], wd_d[:])
        bg = const.tile([128, 4], F32)
        nc.sync.dma_start(bg[:], bg_d[:])
        vo = const.tile([64, 1], F32)
        nc.sync.dma_start(vo[:], vo_d[:])
        idn = const.tile([64, 64], F32)
        nc.sync.dma_start(idn[:], id_d[:])
        if has_bd:
            bd = const.tile([64, VS], F32)
            nc.sync.dma_start(bd[:], bd_d[:])

        h_cur = hpool.tile([128, 8 * 64], F32, name="h_sb")
        nc.sync.dma_start(h_cur[:], h0t[:])
        c_cur = cpool.tile([128, 64], F32, name="c_sb")
        nc.sync.dma_start(c_cur[:], c0t[:])
        xt_cur = xtpool.tile([128, 4 * 64], F32, name="xt_sb")
        nc.sync.dma_start(xt_cur[:], x0t[:])

        # fp32 self-loading matmuls tolerate only one sync wait. Make the PE
        # observe each DMA-loaded tensor it will read via a tiny dummy matmul
        # (one wait each); subsequent matmuls then inherit those clocks.
        wps = dpsum.tile([64, NT], F32, name="dps")
        for src in (wz, wd, idn, h_cur, xt_cur):
            nc.tensor.matmul(
                wps[0:1, 0:1], lhsT=src[0:32, 0:1], rhs=src[0:32, 0:1],
                start=True, stop=True, skip_group_check=True,
            )

        def emit_z_h(zps, h_t):
            # z-chunks k=4..11 (the h @ W_hh part) — only needs gathered h,
            # so these fill the PE while argmax/AG/gather of the previous
            # step run on other engines.
            for m in range(4):
                for ki, k in enumerate(range(4, 12)):
                    nc.tensor.matmul(
                        zps[:, 512 * m : 512 * m + 64],
                        lhsT=wz[:, 512 * k + 128 * m : 512 * k + 128 * (m + 1)],
                        rhs=h_t[:, 64 * (k - 4) : 64 * (k - 3)],
                        start=(ki == 0),
                        stop=False,
                    )

        def emit_z_x(zps, xt):
            # z-chunks k=0..3 (the x @ W_ih part) — after the gather+transpose.
            for m in range(4):
                for ki, k in enumerate(range(4)):
                    nc.tensor.matmul(
                        zps[:, 512 * m : 512 * m + 64],
                        lhsT=wz[:, 512 * k + 128 * m : 512 * k + 128 * (m + 1)],
                        rhs=xt[:, 64 * k : 64 * (k + 1)],
                        start=False,
                        stop=(ki == 3),
                    )

        zps_cur = zpsum.tile([128, 4 * 512], F32, name="zps")
        emit_z_h(zps_cur, h_cur)

        for t in range(T):
            zps = zps_cur
            emit_z_x(zps, xt_cur)

            # ---- LSTM cell elementwise (gate order i,f,g,o at m=0..3) ----
            # sigmoid(x) = 0.5 + 0.5 tanh(x/2); biases folded into ACT bias.
            def act_gate(name, m, scale):
                tl = gates.tile([128, 64], F32, name=name)
                kw = {}
                if has_bg:
                    kw["bias"] = bg[:, m : m + 1]
                nc.scalar.activation(
                    tl[:], zps[:, 512 * m : 512 * m + 64], TANH, scale=scale, **kw
                )
                return tl

            ti = act_gate("ti", 0, 0.5)
            tf = act_gate("tf", 1, 0.5)
            tg = act_gate("tg", 2, 1.0)
            to = act_gate("to", 3, 0.5)

            sf = gates.tile([128, 64], F32, name="sf")  # sigmoid(f) = 0.5+0.5*tf
            nc.vector.tensor_scalar(sf[:], tf[:], 0.5, 0.5, OP.mult, OP.add)
            si = gates.tile([128, 64], F32, name="si")
            nc.vector.tensor_scalar(si[:], ti[:], 0.5, 0.5, OP.mult, OP.add)
            so = gates.tile([128, 64], F32, name="so")
            nc.vector.tensor_scalar(so[:], to[:], 0.5, 0.5, OP.mult, OP.add)
            q1 = gates.tile([128, 64], F32, name="q1")  # sig(f)*c
            nc.vector.tensor_mul(q1[:], sf[:], c_cur[:])
            q2 = gates.tile([128, 64], F32, name="q2")  # sig(i)*tanh(g)
            nc.vector.tensor_mul(q2[:], si[:], tg[:])
            c_new = cpool.tile([128, 64], F32, name="c_sb")
            nc.vector.tensor_add(c_new[:], q1[:], q2[:])
            c_cur = c_new
            tcn = gates.tile([128, 64], F32, name="tcn")  # tanh(c_new)
            nc.scalar.activation(tcn[:], c_new[:], TANH)
            hnew = gates.tile([128, 64], F32, name="hnew")
            nc.vector.tensor_mul(hnew[:], so[:], tcn[:])

            # ---- AllGather h slices -> full h.T on every core ----
            hsl = dram.tile([128, 64], F32, name="hsl")
            nc.sync.dma_start(hsl[:], hnew[:])
            hall = dram.tile([NC * 128, 64], F32, name="hall", addr_space="Shared")
            nc.gpsimd.collective_compute(
                "AllGather",
                OP.bypass,
                replica_groups=RG,
                ins=[hsl[:].opt()],
                outs=[hall[:].opt()],
            )
            h_new_sb = hpool.tile([128, 8 * 64], F32, name="h_sb")
            nc.sync.dma_start(
                h_new_sb[:].rearrange("p (u b) -> p u b", b=64),
                hall[:].rearrange("(u p) b -> p u b", p=128),
            )
            h_cur = h_new_sb

            # ---- dense: logits[64, 4000] = h.T^T @ Wd (+ b_dense) ----
            logits = lpool.tile([64, VS], F32, name="logits")
            HV = VS // 2
            if t < T - 1:
                lmaxA = ampool.tile([64, 8], F32, name="lmaxA")
                lidxA = ampool.tile([64, 8], U32, name="lidxA")
            for n in range(NTILES):
                dps = dpsum.tile([64, NT], F32, name="dps")
                for k in range(KD):
                    nc.tensor.matmul(
                        dps[:],
                        lhsT=h_cur[:, 64 * k : 64 * (k + 1)],
                        rhs=wd[:, VS * k + NT * n : VS * k + NT * (n + 1)],
                        start=(k == 0),
                        stop=(k == KD - 1),
                    )
                if has_bd:
                    nc.vector.tensor_add(
                        logits[:, NT * n : NT * (n + 1)], dps[:], bd[:, NT * n : NT * (n + 1)]
                    )
                else:
                    nc.vector.tensor_copy(logits[:, NT * n : NT * (n + 1)], dps[:])
                if t < T - 1 and (n + 1) * NT == HV:
                    # first-half argmax hides under the second half of dense
                    nc.vector.max(out=lmaxA[:], in_=logits[:, :HV])
                    nc.vector.max_index(lidxA[:], lmaxA[:], logits[:, :HV])

            nc.sync.dma_start(out_d[t], logits[:])

            if t == T - 1:
                break

            # next step's h-part matmuls fill the PE during argmax/AG/gather
            zps_cur = zpsum.tile([128, 4 * 512], F32, name="zps")
            emit_z_h(zps_cur, h_cur)

            # ---- local top-1 (second half + merge) + global argmax combine ----
            lmaxB = ampool.tile([64, 8], F32, name="lmaxB")
            nc.vector.max(out=lmaxB[:], in_=logits[:, HV:])
            lidxB = ampool.tile([64, 8], U32, name="lidxB")
            nc.vector.max_index(lidxB[:], lmaxB[:], logits[:, HV:])
            idxfA = ampool.tile([64, 1], F32, name="idxfA")
            nc.vector.tensor_copy(idxfA[:], lidxA[:, 0:1])
            idxfB = ampool.tile([64, 1], F32, name="idxfB")
            nc.vector.tensor_scalar(idxfB[:], lidxB[:, 0:1], float(HV), None, OP.add)
            cge = ampool.tile([64, 1], U32, name="cge")
            nc.vector.tensor_tensor(
                out=cge[:], in0=lmaxA[:, 0:1], in1=lmaxB[:, 0:1], op=OP.is_ge
            )
            lpick = ampool.tile([64, 1], F32, name="lpick")
            nc.vector.tensor_copy(lpick[:], idxfB[:])
            nc.vector.copy_predicated(lpick[:], cge[:], idxfA[:])
            pk = ampool.tile([64, 2], F32, name="pk")
            nc.vector.tensor_tensor(
                out=pk[:, 0:1], in0=lmaxA[:, 0:1], in1=lmaxB[:, 0:1], op=OP.max
            )
            nc.vector.tensor_add(pk[:, 1:2], lpick[:], vo[:])

            amin = dram.tile([64, 2], F32, name="amin")
            nc.sync.dma_start(amin[:], pk[:])
            amout = dram.tile([NC * 64, 2], F32, name="amout", addr_space="Shared")
            nc.gpsimd.collective_compute(
                "AllGather",
                OP.bypass,
                replica_groups=RG,
                ins=[amin[:].opt()],
                outs=[amout[:].opt()],
            )
            cand = ampool.tile([64, 16], F32, name="cand")
            nc.sync.dma_start(
                cand[:].rearrange("b (c j) -> b c j", j=2),
                amout[:].rearrange("(c b) j -> b c j", c=NC),
            )
            c3 = cand[:].rearrange("b (c j) -> b c j", j=2)
            vals = c3[:, :, 0]
            idxs = c3[:, :, 1]
            gmx = ampool.tile([64, 1], F32, name="gmx")
            nc.vector.tensor_reduce(gmx[:], vals, axis=mybir.AxisListType.X, op=OP.max)
            eq = ampool.tile([64, 8], U32, name="eq")
            nc.vector.tensor_tensor(
                out=eq[:], in0=vals, in1=gmx[:].to_broadcast([64, 8]), op=OP.is_equal
            )
            pick = ampool.tile([64, 8], F32, name="pick")
            nc.vector.memset(pick[:], BIG)
            nc.vector.copy_predicated(pick[:], eq[:], idxs)
            gixf = ampool.tile([64, 1], F32, name="gixf")
            nc.vector.tensor_reduce(gixf[:], pick[:], axis=mybir.AxisListType.X, op=OP.min)
            gi32 = ampool.tile([64, 1], I32, name="gi32")
            nc.vector.tensor_copy(gi32[:], gixf[:])

            # ---- gather emb[idx] and transpose to [E-part, batch] blocks ----
            xr = xpool.tile([64, E], F32, name="xr")
            nc.gpsimd.indirect_dma_start(
                out=xr[:],
                out_offset=None,
                in_=emb_d[:],
                in_offset=bass.IndirectOffsetOnAxis(ap=gi32[:, :1], axis=0),
            )
            xt_next = xtpool.tile([128, 4 * 64], F32, name="xt_sb")
            for j in range(4):
                tps = tpsum.tile([128, 64], F32, name="tps")
                nc.tensor.transpose(
                    tps[:], xr[:, 128 * j : 128 * (j + 1)], idn[:]
                )
                nc.vector.tensor_copy(xt_next[:, 64 * j : 64 * (j + 1)], tps[:])
            xt_cur = xt_next

    nc.compile()
    return nc


def make_in_maps(inputs: dict, T: int = T_FULL):
    h0 = np.ascontiguousarray(np.asarray(inputs["h0"], np.float32))
    c0 = np.ascontiguousarray(np.asarray(inputs["c0"], np.float32))
    emb = np.ascontiguousarray(np.asarray(inputs["emb"], np.float32))
    W_ih = np.asarray(inputs["W_ih"], np.float32)
    W_hh = np.asarray(inputs["W_hh"], np.float32)
    b = np.asarray(inputs["b"], np.float32)
    W_d = np.asarray(inputs["W_dense"], np.float32)
    b_d = np.asarray(inputs["b_dense"], np.float32)

    has_bd = bool(np.any(b_d != 0))
    has_bg = bool(np.any(b != 0))

    Wz_full = np.concatenate([W_ih, W_hh], axis=0)  # [1536, 4096]
    h0t = np.ascontiguousarray(
        h0.T.reshape(8, 128, 64).transpose(1, 0, 2).reshape(128, 512)
    )
    x0 = emb[GO]  # [512]
    x0t = np.ascontiguousarray(
        np.repeat(x0[:, None], B, axis=1).reshape(4, 128, 64).transpose(1, 0, 2).reshape(128, 256)
    )
    ident = np.eye(64, dtype=np.float32)

    in_maps = []
    for c in range(NC):
        ucols = np.concatenate(
            [np.arange(g * U + 128 * c, g * U + 128 * (c + 1)) for g in range(4)]
        )
        Wz_c = Wz_full[:, ucols]  # [1536, 512] cols: 128 i, 128 f, 128 g, 128 o
        # reorder cols so gate-block m at [:,128m:128(m+1)] -> already true
        wz_l = np.ascontiguousarray(
            Wz_c.reshape(KZ, 128, 512).transpose(1, 0, 2).reshape(128, KZ * 512)
        )
        Wd_c = W_d[:, VS * c : VS * (c + 1)]  # [1024, 4000]
        wd_l = np.ascontiguousarray(
            Wd_c.reshape(KD, 128, VS).transpose(1, 0, 2).reshape(128, KD * VS)
        )
        bgate = np.zeros((128, 4), np.float32)
        for g, scale in zip(range(4), (0.5, 0.5, 1.0, 0.5)):
            bgate[:, g] = b[g * U + 128 * c : g * U + 128 * (c + 1)] * scale
        c0t = np.ascontiguousarray(c0[:, 128 * c : 128 * (c + 1)].T)
        vocoff = np.full((64, 1), VS * c, np.float32)
        m = {
            "h0t": h0t,
            "c0t": c0t,
            "emb": emb,
            "x0t": x0t,
            "wz": wz_l,
            "wd": wd_l,
            "bgate": bgate,
            "vocoff": vocoff,
            "ident": ident,
        }
        if has_bd:
            m["bd"] = np.ascontiguousarray(
                np.repeat(b_d[VS * c : VS * (c + 1)][None, :], B, axis=0)
            )
        in_maps.append(m)
    return in_maps, has_bd, has_bg


def assemble_output(results, T: int = T_FULL):
    parts = [np.asarray(r["out"]).reshape(T, B, VS) for r in results]
    full = np.concatenate(parts, axis=2)  # [T, 64, 32000]
    return np.ascontiguousarray(full.transpose(1, 0, 2))  # [64, T, 32000]


def kernel(**inputs) -> np.ndarray:
    in_maps, has_bd, has_bg = make_in_maps(inputs)
    nc = build_program(T_FULL, has_bd=has_bd, has_bg=has_bg)
    res = run_bass_kernel_spmd(nc, in_maps, core_ids=list(range(NC)))
    return assemble_output(res.results)


if __name__ == "__main__":
    # smoke: random small check against numpy oracle is in test.py
    print("kernel module OK")


# revision 15
# speedup vs baseline: 1.4711x; 1.2511x over previous
"""Trainium2 Bass kernel for nn_ChatDecoder: greedy LSTM decoder, 32 steps.

Strategy (8 NeuronCores, SPMD):
  - Vocab-sharded dense projection: each core holds its W_dense[:, c*4000:(c+1)*4000]
    slice resident in SBUF (as an fp16 two-term split: W1=fp16(W),
    W2=fp16((W-W1)*2^11)) and computes logits [64, 4000] per step with
    fp32-quality accuracy at fp16 matmul speed:
      logits = 2^-11 * (A1s@W1 + A2@W1 + A1@W2),  A1=fp16(h), A1s=A1*2^11,
      A2=fp16((h-A1)*2^11) — all three accumulate in one PSUM bank.
  - Unit-sharded LSTM: each core computes 128 of the 1024 hidden units
    (all four gates, batch-major [64, 128] tiles), then an AllGather
    assembles full h.T on every core (8 per-block DMAs back to SBUF).
  - The x @ W_ih + b path is folded into a host-precomputed (float64) table
    videmb = emb @ W_ih[:, own-cols] + b[own-cols]  [32000, 512] per core;
    each step indirect-DMA-gathers videmb[idx] -> zx, so only h @ W_hh runs
    on the PE (8 wide fp32 matmuls, emitted early so they overlap the
    previous step's argmax/AllGather/gather chain).
  - Greedy argmax: two-half DVE max/max_index (first half hidden under the
    dense), tiny AllGather of (value, global index), local combine with
    first-occurrence tie-break matching jnp.argmax.
  - Gates use tanh only: sigmoid(x) = 0.5 + 0.5*tanh(x/2) (tanh's table is
    ~10x more accurate than sigmoid's; greedy decode here has argmax
    margins down to 1.4e-6, so everything on the argmax path is kept at
    fp32-or-better accuracy).

Output per core: [32, 64, 4000] (step, batch, vocab shard); host concatenates
shards and transposes to [64, 32, 32000].
"""

import sys
from contextlib import ExitStack

import numpy as np

for _p in ("/opt/trn_rl_repo",):
    if _p not in sys.path:
        sys.path.insert(0, _p)

import concourse.bass as bass
import concourse.tile as tile
from concourse import bacc, mybir
from concourse.bass_utils import run_bass_kernel_spmd

F32 = mybir.dt.float32
F16 = mybir.dt.float16
I32 = mybir.dt.int32
U32 = mybir.dt.uint32
TANH = mybir.ActivationFunctionType.Tanh
OP = mybir.AluOpType

V, E, U, B, T_FULL = 32000, 512, 1024, 64, 32
NC = 8
VS = V // NC          # 4000 vocab shard
NT = 500              # dense moving tile (<=512)
NTILES = VS // NT     # 8
KD = U // 128         # 8 dense K-chunks
GO = 1
RG = [list(range(NC))]
SC = 2048.0           # 2^11 split scale


def build_program(T: int = T_FULL, has_bd: bool = False):
    nc = bacc.Bacc(
        "TRN2", target_bir_lowering=False, debug=False, num_devices=NC
    )

    def inp(name, shape, dtype=F32):
        return nc.dram_tensor(name, list(shape), dtype, kind="ExternalInput")

    h0t = inp("h0t", (128, 8 * 64))
    c0 = inp("c0", (64, 128))
    videmb_d = inp("videmb", (V, 512))
    zx0_d = inp("zx0", (64, 512))
    whh_d = inp("whh", (128, 8 * 512))
    wd1_d = inp("wd1", (128, KD * VS), F16)
    wd2_d = inp("wd2", (128, KD * VS), F16)
    vo_d = inp("vocoff", (64, 1))
    id_d = inp("ident", (64, 64))
    if has_bd:
        bd_d = inp("bd", (64, VS))
    out_d = nc.dram_tensor("out", [T, B, VS], F32, kind="ExternalOutput")

    with tile.TileContext(nc) as tc, ExitStack() as ctx:
        const = ctx.enter_context(tc.tile_pool(name="const", bufs=1))
        hpool = ctx.enter_context(tc.tile_pool(name="hpool", bufs=2))
        spool = ctx.enter_context(tc.tile_pool(name="spool", bufs=2))
        cpool = ctx.enter_context(tc.tile_pool(name="cpool", bufs=2))
        zxpool = ctx.enter_context(tc.tile_pool(name="zxpool", bufs=2))
        gates = ctx.enter_context(tc.tile_pool(name="gates", bufs=2))
        lpool = ctx.enter_context(tc.tile_pool(name="lpool", bufs=1))
        ampool = ctx.enter_context(tc.tile_pool(name="ampool", bufs=2))
        dram = ctx.enter_context(tc.tile_pool(name="dram", bufs=2, space="DRAM"))
        zpsum = ctx.enter_context(tc.tile_pool(name="zpsum", bufs=1, space="PSUM"))
        dpsum = ctx.enter_context(tc.tile_pool(name="dpsum", bufs=3, space="PSUM"))
        tpsum = ctx.enter_context(tc.tile_pool(name="tpsum", bufs=2, space="PSUM"))

        whh = const.tile([128, 8 * 512], F32)
        nc.sync.dma_start(whh[:], whh_d[:])
        wd1 = const.tile([128, KD * VS], F16)
        nc.sync.dma_start(wd1[:], wd1_d[:])
        wd2 = const.tile([128, KD * VS], F16)
        nc.sync.dma_start(wd2[:], wd2_d[:])
        vo = const.tile([64, 1], F32)
        nc.sync.dma_start(vo[:], vo_d[:])
        idn = const.tile([64, 64], F32)
        nc.sync.dma_start(idn[:], id_d[:])
        if has_bd:
            bd = const.tile([64, VS], F32)
            nc.sync.dma_start(bd[:], bd_d[:])

        h_cur = hpool.tile([128, 8 * 64], F32, name="h_sb")
        nc.sync.dma_start(h_cur[:], h0t[:])
        c_cur = cpool.tile([64, 128], F32, name="c_sb")
        nc.sync.dma_start(c_cur[:], c0[:])
        zx_cur = zxpool.tile([64, 512], F32, name="zx_sb")
        nc.sync.dma_start(zx_cur[:], zx0_d[:])

        # h fp16 split tiles for the dense (produced per step, per K-chunk)
        def split_tiles():
            a1 = spool.tile([128, 8 * 64], F16, name="a1")
            a1s = spool.tile([128, 8 * 64], F16, name="a1s")
            a2 = spool.tile([128, 8 * 64], F16, name="a2")
            return a1, a1s, a2

        def emit_split_chunk(h_t, sp, u):
            a1, a1s, a2 = sp
            s = slice(64 * u, 64 * (u + 1))
            nc.vector.tensor_copy(a1[:, s], h_t[:, s])                 # fp16(h)
            nc.vector.tensor_scalar_mul(a1s[:, s], a1[:, s], SC)       # exact
            tmp = gates.tile([128, 64], F32, name="sptmp")
            nc.vector.tensor_sub(tmp[:], h_t[:, s], a1[:, s])
            nc.vector.tensor_scalar_mul(a2[:, s], tmp[:], SC)

        # fp32 self-loading matmuls tolerate only one sync wait; make the PE
        # observe each DMA-loaded tensor it reads via tiny dummy matmuls.
        wps = dpsum.tile([64, NT], F32, name="dps")
        for src in (whh, wd1, wd2, idn, h_cur):
            nc.tensor.matmul(
                wps[0:1, 0:1], lhsT=src[0:32, 0:1], rhs=src[0:32, 0:1],
                start=True, stop=True, skip_group_check=True,
            )

        def emit_z_h(zps, h_t):
            # z_h[64, 512] = h @ W_hh[:, own cols]; fills the PE while the
            # previous step's argmax/AG/gather chain runs on other engines.
            for k in range(8):
                nc.tensor.matmul(
                    zps[:],
                    lhsT=h_t[:, 64 * k : 64 * (k + 1)],
                    rhs=whh[:, 512 * k : 512 * (k + 1)],
                    start=(k == 0),
                    stop=(k == 7),
                )

        zps_cur = zpsum.tile([64, 512], F32, name="zps")
        emit_z_h(zps_cur, h_cur)

        for t in range(T):
            zps = zps_cur
            zx = zx_cur

            # ---- z = z_h + (x @ W_ih + b)  [gathered] ----
            z_sb = gates.tile([64, 512], F32, name="z_sb")
            nc.vector.tensor_add(z_sb[:], zps[:], zx[:])

            # ---- LSTM cell, batch-major [64, 128] (gates i,f,g,o) ----
            def act_gate(name, g, scale):
                tl = gates.tile([64, 128], F32, name=name)
                nc.scalar.activation(
                    tl[:], z_sb[:, 128 * g : 128 * (g + 1)], TANH, scale=scale
                )
                return tl

            ti = act_gate("ti", 0, 0.5)
            tf = act_gate("tf", 1, 0.5)
            tg = act_gate("tg", 2, 1.0)
            to = act_gate("to", 3, 0.5)

            sf = gates.tile([64, 128], F32, name="sf")
            nc.vector.tensor_scalar(sf[:], tf[:], 0.5, 0.5, OP.mult, OP.add)
            si = gates.tile([64, 128], F32, name="si")
            nc.vector.tensor_scalar(si[:], ti[:], 0.5, 0.5, OP.mult, OP.add)
            so = gates.tile([64, 128], F32, name="so")
            nc.vector.tensor_scalar(so[:], to[:], 0.5, 0.5, OP.mult, OP.add)
            q1 = gates.tile([64, 128], F32, name="q1")
            nc.vector.tensor_mul(q1[:], sf[:], c_cur[:])
            q2 = gates.tile([64, 128], F32, name="q2")
            nc.vector.tensor_mul(q2[:], si[:], tg[:])
            c_new = cpool.tile([64, 128], F32, name="c_sb")
            nc.vector.tensor_add(c_new[:], q1[:], q2[:])
            c_cur = c_new
            tcn = gates.tile([64, 128], F32, name="tcn")
            nc.scalar.activation(tcn[:], c_new[:], TANH)
            hnew = gates.tile([64, 128], F32, name="hnew")
            nc.vector.tensor_mul(hnew[:], so[:], tcn[:])

            # ---- transpose h slice to [128, 64], AllGather full h.T ----
            tph = tpsum.tile([128, 64], F32, name="tph")
            nc.tensor.transpose(tph[:], hnew[:], idn[:])
            hT = gates.tile([128, 64], F32, name="hT")
            nc.vector.tensor_copy(hT[:], tph[:])
            hsl = dram.tile([128, 64], F32, name="hsl")
            nc.sync.dma_start(hsl[:], hT[:])
            hall = dram.tile([NC * 128, 64], F32, name="hall", addr_space="Shared")
            nc.gpsimd.collective_compute(
                "AllGather",
                OP.bypass,
                replica_groups=RG,
                ins=[hsl[:].opt()],
                outs=[hall[:].opt()],
            )
            h_new_sb = hpool.tile([128, 8 * 64], F32, name="h_sb")
            sp = split_tiles()
            for u in range(8):
                nc.sync.dma_start(
                    h_new_sb[:, 64 * u : 64 * (u + 1)],
                    hall[128 * u : 128 * (u + 1), :],
                )
                emit_split_chunk(h_new_sb, sp, u)
            h_cur = h_new_sb
            a1, a1s, a2 = sp

            # ---- dense: logits = 2^-11 (A1s@W1 + A2@W1 + A1@W2) ----
            logits = lpool.tile([64, VS], F32, name="logits")
            HV = VS // 2
            if t < T - 1:
                lmaxA = ampool.tile([64, 8], F32, name="lmaxA")
                lidxA = ampool.tile([64, 8], U32, name="lidxA")
            for n in range(NTILES):
                pr = dpsum.tile([64, NT], F32, name="dps")
                for lhs, w, st, sp_ in (
                    (a1s, wd1, True, False),
                    (a2, wd1, False, False),
                    (a1, wd2, False, True),
                ):
                    for k in range(KD):
                        nc.tensor.matmul(
                            pr[:],
                            lhsT=lhs[:, 64 * k : 64 * (k + 1)],
                            rhs=w[:, VS * k + NT * n : VS * k + NT * (n + 1)],
                            start=(st and k == 0),
                            stop=(sp_ and k == KD - 1),
                        )
                if has_bd:
                    tmpl = gates.tile([64, NT], F32, name="tmpl")
                    nc.vector.tensor_scalar_mul(tmpl[:], pr[:], 1.0 / SC)
                    nc.vector.tensor_add(
                        logits[:, NT * n : NT * (n + 1)], tmpl[:], bd[:, NT * n : NT * (n + 1)]
                    )
                else:
                    nc.vector.tensor_scalar_mul(
                        logits[:, NT * n : NT * (n + 1)], pr[:], 1.0 / SC
                    )
                if t < T - 1 and (n + 1) * NT == HV:
                    nc.vector.max(out=lmaxA[:], in_=logits[:, :HV])
                    nc.vector.max_index(lidxA[:], lmaxA[:], logits[:, :HV])

            nc.sync.dma_start(out_d[t], logits[:])

            if t == T - 1:
                break

            # next step's h-part matmuls fill the PE during argmax/AG/gather
            zps_cur = zpsum.tile([64, 512], F32, name="zps")
            emit_z_h(zps_cur, h_cur)

            # ---- local top-1 (second half + merge) ----
            lmaxB = ampool.tile([64, 8], F32, name="lmaxB")
            nc.vector.max(out=lmaxB[:], in_=logits[:, HV:])
            lidxB = ampool.tile([64, 8], U32, name="lidxB")
            nc.vector.max_index(lidxB[:], lmaxB[:], logits[:, HV:])
            idxfA = ampool.tile([64, 1], F32, name="idxfA")
            nc.vector.tensor_copy(idxfA[:], lidxA[:, 0:1])
            idxfB = ampool.tile([64, 1], F32, name="idxfB")
            nc.vector.tensor_scalar(idxfB[:], lidxB[:, 0:1], float(HV), None, OP.add)
            cge = ampool.tile([64, 1], U32, name="cge")
            nc.vector.tensor_tensor(
                out=cge[:], in0=lmaxA[:, 0:1], in1=lmaxB[:, 0:1], op=OP.is_ge
            )
            lpick = ampool.tile([64, 1], F32, name="lpick")
            nc.vector.tensor_copy(lpick[:], idxfB[:])
            nc.vector.copy_predicated(lpick[:], cge[:], idxfA[:])
            pk = ampool.tile([64, 2], F32, name="pk")
            nc.vector.tensor_tensor(
                out=pk[:, 0:1], in0=lmaxA[:, 0:1], in1=lmaxB[:, 0:1], op=OP.max
            )
            nc.vector.tensor_add(pk[:, 1:2], lpick[:], vo[:])

            # ---- global argmax combine via tiny AllGather ----
            amin = dram.tile([64, 2], F32, name="amin")
            nc.sync.dma_start(amin[:], pk[:])
            amout = dram.tile([NC * 64, 2], F32, name="amout", addr_space="Shared")
            nc.gpsimd.collective_compute(
                "AllGather",
                OP.bypass,
                replica_groups=RG,
                ins=[amin[:].opt()],
                outs=[amout[:].opt()],
            )
            cand = ampool.tile([64, 16], F32, name="cand")
            nc.sync.dma_start(
                cand[:].rearrange("b (c j) -> b c j", j=2),
                amout[:].rearrange("(c b) j -> b c j", c=NC),
            )
            c3 = cand[:].rearrange("b (c j) -> b c j", j=2)
            vals = c3[:, :, 0]
            idxs = c3[:, :, 1]
            gmx = ampool.tile([64, 1], F32, name="gmx")
            nc.vector.tensor_reduce(gmx[:], vals, axis=mybir.AxisListType.X, op=OP.max)
            eq = ampool.tile([64, 8], U32, name="eq")
            nc.vector.tensor_tensor(
                out=eq[:], in0=vals, in1=gmx[:].to_broadcast([64, 8]), op=OP.is_equal
            )
            pick = ampool.tile([64, 8], F32, name="pick")
            nc.vector.memset(pick[:], 1.0e9)
            nc.vector.copy_predicated(pick[:], eq[:], idxs)
            gixf = ampool.tile([64, 1], F32, name="gixf")
            nc.vector.tensor_reduce(gixf[:], pick[:], axis=mybir.AxisListType.X, op=OP.min)
            gi32 = ampool.tile([64, 1], I32, name="gi32")
            nc.vector.tensor_copy(gi32[:], gixf[:])

            # ---- gather next step's x-side pre-activations ----
            zx_next = zxpool.tile([64, 512], F32, name="zx_sb")
            nc.gpsimd.indirect_dma_start(
                out=zx_next[:],
                out_offset=None,
                in_=videmb_d[:],
                in_offset=bass.IndirectOffsetOnAxis(ap=gi32[:, :1], axis=0),
            )
            zx_cur = zx_next

    nc.compile()
    return nc


def make_in_maps(inputs: dict, T: int = T_FULL):
    h0 = np.ascontiguousarray(np.asarray(inputs["h0"], np.float32))
    c0 = np.ascontiguousarray(np.asarray(inputs["c0"], np.float32))
    emb = np.ascontiguousarray(np.asarray(inputs["emb"], np.float32))
    W_ih = np.asarray(inputs["W_ih"], np.float32)
    W_hh = np.asarray(inputs["W_hh"], np.float32)
    b = np.asarray(inputs["b"], np.float32)
    W_d = np.asarray(inputs["W_dense"], np.float32)
    b_d = np.asarray(inputs["b_dense"], np.float32)

    has_bd = bool(np.any(b_d != 0))

    h0t = np.ascontiguousarray(
        h0.T.reshape(8, 128, 64).transpose(1, 0, 2).reshape(128, 512)
    )
    ident = np.eye(64, dtype=np.float32)

    # videmb = emb @ W_ih + b in float64, per-core column slice
    emb64 = emb.astype(np.float64)
    Wih64 = W_ih.astype(np.float64)
    b64 = b.astype(np.float64)

    in_maps = []
    for c in range(NC):
        ucols = np.concatenate(
            [np.arange(g * U + 128 * c, g * U + 128 * (c + 1)) for g in range(4)]
        )
        videmb = (emb64 @ Wih64[:, ucols] + b64[ucols]).astype(np.float32)
        zx0 = np.ascontiguousarray(np.repeat(videmb[GO][None, :], B, axis=0))
        Whh_c = W_hh[:, ucols]  # [1024, 512]
        whh_l = np.ascontiguousarray(
            Whh_c.reshape(8, 128, 512).transpose(1, 0, 2).reshape(128, 8 * 512)
        )
        Wd_c = W_d[:, VS * c : VS * (c + 1)]  # [1024, 4000]
        W1 = Wd_c.astype(np.float16)
        W2 = ((Wd_c - W1.astype(np.float32)) * SC).astype(np.float16)
        lay16 = lambda M: np.ascontiguousarray(
            M.reshape(KD, 128, VS).transpose(1, 0, 2).reshape(128, KD * VS)
        )
        c0_c = np.ascontiguousarray(c0[:, 128 * c : 128 * (c + 1)])
        vocoff = np.full((64, 1), VS * c, np.float32)
        m = {
            "h0t": h0t,
            "c0": c0_c,
            "videmb": videmb,
            "zx0": zx0,
            "whh": whh_l,
            "wd1": lay16(W1),
            "wd2": lay16(W2),
            "vocoff": vocoff,
            "ident": ident,
        }
        if has_bd:
            m["bd"] = np.ascontiguousarray(
                np.repeat(b_d[VS * c : VS * (c + 1)][None, :], B, axis=0)
            )
        in_maps.append(m)
    return in_maps, has_bd, False


def assemble_output(results, T: int = T_FULL):
    parts = [np.asarray(r["out"]).reshape(T, B, VS) for r in results]
    full = np.concatenate(parts, axis=2)  # [T, 64, 32000]
    return np.ascontiguousarray(full.transpose(1, 0, 2))  # [64, T, 32000]


def kernel(**inputs) -> np.ndarray:
    in_maps, has_bd, _ = make_in_maps(inputs)
    nc = build_program(T_FULL, has_bd=has_bd)
    res = run_bass_kernel_spmd(nc, in_maps, core_ids=list(range(NC)))
    return assemble_output(res.results)


if __name__ == "__main__":
    print("kernel module OK")
